# revision 21
# baseline (speedup 1.0000x reference)
"""BiRWKV layer kernel for 8 Trainium2 NeuronCores.

Strategy (data-parallel over B=8, one batch element per core):
  - (channel, time) layout on chip: channels on the 128 SBUF partitions
    (C=512 -> 4 blocks), time on the free dim. Full-T (4096) stripe
    arrays per (direction, channel-block).
  - r/k/v projections are bf16 matmuls (lhsT = W block, rhs = x^T block)
    accumulated over 4 input-channel blocks into PSUM (fp32); x^T is
    fully SBUF-resident so weight blocks are reused across 2-wide
    time-tile groups.
  - WKV runs UNSTABILIZED (mathematically equal to the reference's
    log-sum-exp form; values stay in range since |w|*T <= ~28, k~N(0,1)):
        den_t = d*den_{t-1} + e^{k_t};  num_t = d*num_{t-1} + e^{k_t} v_t
        y_t   = (num_{t-1} + E e^{k_t} v_t) / (den_{t-1} + E e^{k_t}),
    with E = e^u folded in as a per-partition scalar_tensor_tensor
    scalar (no exp(k+u) activation needed).
  - den/num recurrences: DVE tensor_tensor_scan, 2 chained 2048-wide
    instructions per variable sharing one (128, 4097) chain tile (the
    second half's init reads the first half's last element in place).
    Backward direction uses reversed access patterns.
  - y = (num_prev + ekb) / (den_prev + ekb*v...) with ekb = e^{k+u} from
    an Exp ACT with per-partition bias; ekv = ekbv * e^{-u} via a
    Copy-ACT with per-partition scale. dy/ny adds and ekbv mult run on
    GpSimd (SBUF-only tensor_tensor); the divide is vector.reciprocal
    (bf16) + three 2x-mode DVE tensor_tensor ops for q, q*th, and
    y = q + q*th. 0.5 of the sigmoid is folded into W_out on the host.
  - Scalar engine only runs Exp/Tanh/Copy (one ACT table, no reloads).
  - Output projection keeps W_out blocks as matmul lhsT (stationary)
    and y tiles (channel, time) as rhs; result is (C, T) in PSUM ->
    SBUF -> HBM, transposed to (T, C) on the host.
"""

import numpy as np
import ml_dtypes

B, T, C = 8, 4096, 512
TT = 512           # time tile (psum width)
NTT = T // TT      # 8
CB = 4             # channel blocks
HW = 2048          # scan half width
CHK = 1024         # y-stage chunk width
NCHK = T // CHK    # 4

_CACHE = {}


def _apply_tile_patches():
    """walrus in this container rejects instructions with >1 sync wait
    ("Too many sync wait commands"). Split excess waits onto same-engine
    nop carriers, and do the same for the TileContext tail drain."""
    import concourse.tile as tile_mod
    from concourse import mybir
    from concourse.vector_clock import ScopedClock

    if getattr(tile_mod, "_wait_split_patched", False):
        return
    MAXW = 1

    _orig_add = tile_mod.TileContext._add_instruction

    def _split_add(self, inst):
        si = inst.sync_info
        if si is not None and si.on_wait and len(si.on_wait) > MAXW:
            waits = list(si.on_wait)
            k = 0
            while len(waits) > MAXW:
                chunk, waits = waits[:MAXW], waits[MAXW:]
                carrier = mybir.InstNoOp(
                    name=f"{inst.name}_wsplit{k}",
                    engine=inst.engine,
                    bass_nofuse=True,
                    sync_info=mybir.SyncInfo(on_wait=chunk, on_update=[]),
                )
                k += 1
                _orig_add(self, carrier)
            inst.sync_info = mybir.SyncInfo(
                on_wait=waits, on_update=list(si.on_update)
            )
        return _orig_add(self, inst)

    def _drain_and_barrier(self, tick_clock, wait_clock):
        drain_inst = self.nc.sync.drain()
        wait_clock.add_sem_waits(
            drain_inst.ins, ScopedClock({None: tick_clock.global_clock})
        )
        si = drain_inst.ins.sync_info
        if si is not None and si.on_wait and len(si.on_wait) > MAXW:
            waits = list(si.on_wait)
            drain_inst.ins.sync_info = mybir.SyncInfo(
                on_wait=waits[:MAXW], on_update=list(si.on_update)
            )
            rest = waits[MAXW:]
            while rest:
                chunk, rest = rest[:MAXW], rest[MAXW:]
                n = self.nc.sync.nop(nofuse=True)
                n.ins.sync_info = mybir.SyncInfo(on_wait=chunk, on_update=[])

        self.nc.all_engine_barrier()
        assert self.sems is not None
        popped = self.nc._tile_sem_poison_stack.pop()
        assert popped is self._sem_poison
        self.nc.clear_and_free_semaphores(list(self.sems.allocated().values()))
        self.nc.all_engine_barrier()

    tile_mod.TileContext._add_instruction = _split_add
    tile_mod.TileContext._drain_and_barrier = _drain_and_barrier
    tile_mod._wait_split_patched = True


def _build_nc():
    import concourse.bass as bass
    import concourse.tile as tile
    from concourse import mybir

    _apply_tile_patches()

    f32 = mybir.dt.float32
    bf16 = mybir.dt.bfloat16
    Alu = mybir.AluOpType
    Act = mybir.ActivationFunctionType

    nc = bass.Bass()

    xT = nc.dram_tensor("xT", [C, T], bf16, kind="ExternalInput")
    wnames = ["w_rf", "w_kf", "w_vf", "w_rb", "w_kb", "w_vb"]
    wdram = {
        n: nc.dram_tensor(n, [128, 4 * C], bf16, kind="ExternalInput")
        for n in wnames
    }
    wout_d = nc.dram_tensor("wout", [128, 8 * C], bf16, kind="ExternalInput")
    u_f_d = nc.dram_tensor("u_f", [C, 1], f32, kind="ExternalInput")
    u_b_d = nc.dram_tensor("u_b", [C, 1], f32, kind="ExternalInput")
    eu_f_d = nc.dram_tensor("eu_f", [C, 1], f32, kind="ExternalInput")
    eu_b_d = nc.dram_tensor("eu_b", [C, 1], f32, kind="ExternalInput")
    dec_f_d = nc.dram_tensor("dec_f", [C, 1], f32, kind="ExternalInput")
    dec_b_d = nc.dram_tensor("dec_b", [C, 1], f32, kind="ExternalInput")
    out_d = nc.dram_tensor("yT", [C, T], f32, kind="ExternalOutput")
    yst = {d: nc.dram_tensor(f"yst_{d}", [C, T], bf16) for d in ("f", "b")}

    with tile.TileContext(nc) as tc:
        with (
            tc.tile_pool(name="wp", bufs=1) as wp,
            tc.tile_pool(name="cst", bufs=1) as cst,
            tc.tile_pool(name="xr", bufs=1) as xrp,
            tc.tile_pool(name="arr", bufs=2) as arrp,
            tc.tile_pool(name="chn", bufs=2) as chnp,
            tc.tile_pool(name="yc", bufs=2) as ycp,
            tc.tile_pool(name="op", bufs=2) as opp,
            tc.tile_pool(name="ps", bufs=1, space="PSUM") as psp,
        ):
            # ---- resident weights, x, constants ----
            wout = wp.tile([128, 8 * C], bf16, name="wout")
            nc.sync.dma_start(wout[:], wout_d[:])
            wt = {}
            for n in wnames:
                wt[n] = wp.tile([128, 4 * C], bf16, tag=n, name=n)
                nc.sync.dma_start(wt[n][:], wdram[n][:])
            xts = {}
            for kb in range(4):
                xts[kb] = xrp.tile([128, T], bf16, tag=f"x{kb}", name=f"x{kb}")
                nc.sync.dma_start(xts[kb][:], xT[kb * 128:(kb + 1) * 128, :])
            u_t, eu_t, dec_t = {}, {}, {}
            for cb in range(CB):
                sl = slice(cb * 128, (cb + 1) * 128)
                for d, ud, eud, dd in (("f", u_f_d, eu_f_d, dec_f_d),
                                       ("b", u_b_d, eu_b_d, dec_b_d)):
                    u_t[(d, cb)] = cst.tile([128, 1], f32, tag=f"u{d}{cb}",
                                            name=f"u{d}{cb}")
                    nc.sync.dma_start(u_t[(d, cb)][:], ud[sl, :])
                    eu_t[(d, cb)] = cst.tile([128, 1], f32, tag=f"e{d}{cb}",
                                             name=f"e{d}{cb}")
                    nc.sync.dma_start(eu_t[(d, cb)][:], eud[sl, :])
                    dec_t[(d, cb)] = cst.tile([128, 1], f32, tag=f"d{d}{cb}",
                                              name=f"d{d}{cb}")
                    nc.sync.dma_start(dec_t[(d, cb)][:], dd[sl, :])

            def run_stripe(d, cb):
                fwd = d == "f"
                wr, wk, wv = wt["w_r" + d], wt["w_k" + d], wt["w_v" + d]
                ek = arrp.tile([128, T], bf16, tag="ek", name="ek")
                ekb = arrp.tile([128, T], bf16, tag="ekb", name="ekb")
                vsb = arrp.tile([128, T], bf16, tag="vsb", name="vsb")
                th = arrp.tile([128, T], bf16, tag="th", name="th")
                ekv = arrp.tile([128, T], bf16, tag="ekv", name="ekv")
                chd = chnp.tile([128, T + 1], bf16, tag="chd", name="chd")
                chn = chnp.tile([128, T + 1], bf16, tag="chn", name="chn")
                ub = u_t[(d, cb)][:, 0:1]
                eu = eu_t[(d, cb)][:, 0:1]

                # ---- projections + activations, 2-wide time-tile groups
                for ttg in range(NTT // 2):
                    for cls, w in (("k", wk), ("v", wv), ("r", wr)):
                        pss = {}
                        for h in range(2):
                            pss[h] = psp.tile([128, TT], f32, tag=f"p{cls}",
                                              bufs=2, name=f"p{cls}")
                        for kb in range(4):
                            wsl = w[:, kb * C + cb * 128:
                                    kb * C + cb * 128 + 128]
                            for h in range(2):
                                t0 = (2 * ttg + h) * TT
                                nc.tensor.matmul(
                                    pss[h][:], wsl, xts[kb][:, t0:t0 + TT],
                                    start=(kb == 0), stop=(kb == 3))
                        for h in range(2):
                            t0 = (2 * ttg + h) * TT
                            if cls == "k":
                                nc.scalar.activation(ek[:, t0:t0 + TT],
                                                     pss[h][:], Act.Exp)
                                nc.scalar.activation(ekb[:, t0:t0 + TT],
                                                     pss[h][:], Act.Exp,
                                                     bias=ub)
                            elif cls == "v":
                                nc.scalar.copy(vsb[:, t0:t0 + TT], pss[h][:])
                            else:
                                nc.scalar.activation(th[:, t0:t0 + TT],
                                                     pss[h][:], Act.Tanh,
                                                     bias=0.0, scale=0.5)

                # ---- ekv = ek * v on gpsimd (scan input)
                for c in range(NCHK):
                    cs = slice(c * CHK, (c + 1) * CHK)
                    nc.gpsimd.tensor_mul(ekv[:, cs], ek[:, cs], vsb[:, cs])

                # ---- scans (2 chained 2048-wide per variable)
                decbc = dec_t[(d, cb)][:, 0:1].broadcast_to([128, HW])
                if fwd:
                    nc.vector.memset(chd[:, 0:1], 0.0)
                    nc.vector.memset(chn[:, 0:1], 0.0)
                    for loc in (0, HW):
                        nc.vector.tensor_tensor_scan(
                            chd[:, 1 + loc: 1 + loc + HW], decbc,
                            ek[:, loc: loc + HW], chd[:, loc: loc + 1],
                            Alu.mult, Alu.add)
                        nc.vector.tensor_tensor_scan(
                            chn[:, 1 + loc: 1 + loc + HW], decbc,
                            ekv[:, loc: loc + HW], chn[:, loc: loc + 1],
                            Alu.mult, Alu.add)
                    den_prev = chd[:, 0:T]
                    num_prev = chn[:, 0:T]
                else:
                    nc.vector.memset(chd[:, T:T + 1], 0.0)
                    nc.vector.memset(chn[:, T:T + 1], 0.0)
                    for loc in (HW, 0):
                        nc.vector.tensor_tensor_scan(
                            chd[:, loc: loc + HW][:, ::-1], decbc,
                            ek[:, loc: loc + HW][:, ::-1],
                            chd[:, loc + HW: loc + HW + 1],
                            Alu.mult, Alu.add)
                        nc.vector.tensor_tensor_scan(
                            chn[:, loc: loc + HW][:, ::-1], decbc,
                            ekv[:, loc: loc + HW][:, ::-1],
                            chn[:, loc + HW: loc + HW + 1],
                            Alu.mult, Alu.add)
                    den_prev = chd[:, 1:T + 1]
                    num_prev = chn[:, 1:T + 1]

                # ---- y stage per 1024 chunk
                #   dy = den_prev + e^u ek = den_prev + ekb     [gpsimd]
                #   ny = num_prev + e^u ekv = num_prev + ekbv   [gpsimd]
                #   y  = (ny / dy) * (1 + tanh(r/2))            [DVE 2x]
                # dead slices are recycled as outputs: inv->vsb, q->ekv,
                # q2->ek, y->ekb (each input is fully consumed upstream
                # of the write; the tile framework tracks subtile WAR).
                for c in range(NCHK):
                    cs = slice(c * CHK, (c + 1) * CHK)
                    ekbv = ycp.tile([128, CHK], bf16, tag="ekbv",
                                    name="ekbv")
                    dy = ycp.tile([128, CHK], bf16, tag="dy", name="dy")
                    ny = ycp.tile([128, CHK], bf16, tag="ny", name="ny")
                    inv = vsb[:, cs]
                    q = ekv[:, cs]
                    q2 = ek[:, cs]
                    y = ekb[:, cs]
                    nc.scalar.activation(ekbv[:], ekv[:, cs], Act.Copy,
                                         scale=eu)
                    nc.gpsimd.tensor_add(dy[:], ekb[:, cs], den_prev[:, cs])
                    nc.gpsimd.tensor_add(ny[:], ekbv[:], num_prev[:, cs])
                    with nc.allow_low_precision("den>=exp(k+u)>0; bf16 y"):
                        nc.vector.reciprocal(inv, dy[:])
                    nc.vector.tensor_mul(q, ny[:], inv)
                    nc.vector.tensor_mul(q2, q, th[:, cs])
                    nc.vector.tensor_add(y, q, q2)
                    nc.sync.dma_start(
                        yst[d][cb * 128:(cb + 1) * 128, cs], y)

            for cb in range(CB):
                run_stripe("f", cb)
            for cb in range(CB):
                run_stripe("b", cb)

            # ---- output projection: out^T[c_out, t] in (C, T) layout
            for tc_i in range(NTT):
                t0 = tc_i * TT
                yld = {}
                for j in range(8):
                    dd = "f" if j < 4 else "b"
                    cbj = j % 4
                    yld[j] = opp.tile([128, TT], bf16, tag=f"yl{j}",
                                      name=f"yl{j}")
                    nc.sync.dma_start(
                        yld[j][:],
                        yst[dd][cbj * 128:(cbj + 1) * 128, t0:t0 + TT])
                for cbo in range(CB):
                    pso = psp.tile([128, TT], f32, tag="po", bufs=2,
                                   name="pso")
                    for j in range(8):
                        blk = j * 4 + cbo
                        nc.tensor.matmul(
                            pso[:], wout[:, blk * 128:(blk + 1) * 128],
                            yld[j][:], start=(j == 0), stop=(j == 7))
                    osb = opp.tile([128, TT], f32, tag="osb", bufs=1,
                                   name="osb")
                    nc.scalar.copy(osb[:], pso[:])
                    nc.sync.dma_start(
                        out_d[cbo * 128:(cbo + 1) * 128, t0:t0 + TT], osb[:])

    return nc


def _host_prep(x, W_rkv, W_out, time_decay, time_first, time_decay_rev,
               time_first_rev):
    bf16 = ml_dtypes.bfloat16
    f32 = np.float32

    Wr = W_rkv.reshape(C, 2, 3, C)
    pieces = {
        "w_rf": Wr[:, 0, 0], "w_kf": Wr[:, 0, 1], "w_vf": Wr[:, 0, 2],
        "w_rb": Wr[:, 1, 0], "w_kb": Wr[:, 1, 1], "w_vb": Wr[:, 1, 2],
    }
    wmaps = {}
    for n, p in pieces.items():
        wmaps[n] = np.ascontiguousarray(
            p.reshape(4, 128, C).transpose(1, 0, 2).reshape(128, 4 * C)
        ).astype(bf16)

    # W_out blocks as stationary lhsT: block (j, cbo) = W_out rows
    # j*128:(j+1)*128, cols cbo*128:(cbo+1)*128, laid out at free offset
    # (j*4 + cbo)*128. 0.5 of the sigmoid is folded in.
    Wo = (0.5 * W_out).reshape(8, 128, 4, 128)          # (j, jp, cbo, cp)
    Wo = Wo.transpose(1, 0, 2, 3).reshape(128, 32 * 128)  # (jp, j*4+cbo, cp)
    wout = np.ascontiguousarray(Wo).astype(bf16)

    u_f = np.ascontiguousarray(time_first.reshape(C, 1)).astype(f32)
    u_b = np.ascontiguousarray(time_first_rev.reshape(C, 1)).astype(f32)
    eu_f = np.exp(time_first.astype(np.float64)).reshape(C, 1).astype(f32)
    eu_b = np.exp(time_first_rev.astype(np.float64)).reshape(C, 1).astype(f32)
    dec_f = np.exp(-np.exp(time_decay.astype(np.float64))).reshape(C, 1).astype(f32)
    dec_b = np.exp(-np.exp(time_decay_rev.astype(np.float64))).reshape(C, 1).astype(f32)

    shared = dict(wout=wout, u_f=u_f, u_b=u_b, eu_f=eu_f, eu_b=eu_b,
                  dec_f=dec_f, dec_b=dec_b, **wmaps)
    in_maps = []
    for b in range(B):
        m = dict(shared)
        m["xT"] = np.ascontiguousarray(x[b].T).astype(bf16)
        in_maps.append(m)
    return in_maps


def kernel(x, W_rkv, W_out, time_decay, time_first, time_decay_rev,
           time_first_rev, _trace=False):
    from concourse.bass_utils import run_bass_kernel_spmd

    x = np.asarray(x, dtype=np.float32)
    W_rkv = np.asarray(W_rkv, dtype=np.float32)
    W_out = np.asarray(W_out, dtype=np.float32)
    time_decay = np.asarray(time_decay, dtype=np.float32)
    time_first = np.asarray(time_first, dtype=np.float32)
    time_decay_rev = np.asarray(time_decay_rev, dtype=np.float32)
    time_first_rev = np.asarray(time_first_rev, dtype=np.float32)

    if "nc" not in _CACHE:
        _CACHE["nc"] = _build_nc()
    nc = _CACHE["nc"]

    in_maps = _host_prep(x, W_rkv, W_out, time_decay, time_first,
                         time_decay_rev, time_first_rev)
    res = run_bass_kernel_spmd(
        nc, in_maps, core_ids=list(range(B)), trace=_trace
    )
    _CACHE["last_result"] = res
    out = np.stack([
        np.ascontiguousarray(res.results[b]["yT"].astype(np.float32).T)
        for b in range(B)
    ])
    return out


# revision 26
# speedup vs baseline: 1.2696x; 1.2696x over previous
"""BiRWKV layer kernel for 8 Trainium2 NeuronCores.

Strategy (data-parallel over B=8, one batch element per core):
  - (channel, time) layout on chip: channels on the 128 SBUF partitions
    (C=512 -> 4 blocks), time on the free dim. Full-T (4096) stripe
    arrays per (direction, channel-block).
  - r/k/v projections are bf16 matmuls (lhsT = W block, rhs = x^T block)
    accumulated over 4 input-channel blocks into PSUM (fp32); x^T is
    fully SBUF-resident so weight blocks are reused across 2-wide
    time-tile groups.
  - WKV runs UNSTABILIZED (mathematically equal to the reference's
    log-sum-exp form; values stay in range since |w|*T <= ~28, k~N(0,1)):
        den_t = d*den_{t-1} + e^{k_t};  num_t = d*num_{t-1} + e^{k_t} v_t
        y_t   = (num_{t-1} + E e^{k_t} v_t) / (den_{t-1} + E e^{k_t}),
    with E = e^u folded in as a per-partition scalar_tensor_tensor
    scalar (no exp(k+u) activation needed).
  - den/num recurrences: DVE tensor_tensor_scan, 2 chained 2048-wide
    instructions per variable sharing one (128, 4097) chain tile (the
    second half's init reads the first half's last element in place).
    Backward direction uses reversed access patterns.
  - y = (num_prev + ekb) / (den_prev + ekb*v...) with ekb = e^{k+u} from
    an Exp ACT with per-partition bias; ekv = ekbv * e^{-u} via a
    Copy-ACT with per-partition scale. dy/ny adds and ekbv mult run on
    GpSimd (SBUF-only tensor_tensor); the divide is vector.reciprocal
    (bf16) + three 2x-mode DVE tensor_tensor ops for q, q*th, and
    y = q + q*th. 0.5 of the sigmoid is folded into W_out on the host.
  - Scalar engine only runs Exp/Tanh/Copy (one ACT table, no reloads).
  - Output projection keeps W_out blocks as matmul lhsT (stationary)
    and y tiles (channel, time) as rhs; result is (C, T) in PSUM ->
    SBUF -> HBM, transposed to (T, C) on the host.
"""

import numpy as np
import ml_dtypes

B, T, C = 8, 4096, 512
TT = 512           # time tile (psum width)
NTT = T // TT      # 8
CB = 4             # channel blocks
HW = 2048          # scan half width
CHK = 1024         # y-stage chunk width
NCHK = T // CHK    # 4

_CACHE = {}


def _apply_tile_patches():
    """walrus in this container rejects instructions with >1 sync wait
    ("Too many sync wait commands"). Split excess waits onto same-engine
    nop carriers, and do the same for the TileContext tail drain."""
    import concourse.tile as tile_mod
    from concourse import mybir
    from concourse.vector_clock import ScopedClock

    if getattr(tile_mod, "_wait_split_patched", False):
        return
    MAXW = 1

    _orig_add = tile_mod.TileContext._add_instruction

    def _split_add(self, inst):
        si = inst.sync_info
        if si is not None and si.on_wait and len(si.on_wait) > MAXW:
            waits = list(si.on_wait)
            k = 0
            while len(waits) > MAXW:
                chunk, waits = waits[:MAXW], waits[MAXW:]
                carrier = mybir.InstNoOp(
                    name=f"{inst.name}_wsplit{k}",
                    engine=inst.engine,
                    bass_nofuse=True,
                    sync_info=mybir.SyncInfo(on_wait=chunk, on_update=[]),
                )
                k += 1
                _orig_add(self, carrier)
            inst.sync_info = mybir.SyncInfo(
                on_wait=waits, on_update=list(si.on_update)
            )
        return _orig_add(self, inst)

    def _drain_and_barrier(self, tick_clock, wait_clock):
        drain_inst = self.nc.sync.drain()
        wait_clock.add_sem_waits(
            drain_inst.ins, ScopedClock({None: tick_clock.global_clock})
        )
        si = drain_inst.ins.sync_info
        if si is not None and si.on_wait and len(si.on_wait) > MAXW:
            waits = list(si.on_wait)
            drain_inst.ins.sync_info = mybir.SyncInfo(
                on_wait=waits[:MAXW], on_update=list(si.on_update)
            )
            rest = waits[MAXW:]
            while rest:
                chunk, rest = rest[:MAXW], rest[MAXW:]
                n = self.nc.sync.nop(nofuse=True)
                n.ins.sync_info = mybir.SyncInfo(on_wait=chunk, on_update=[])

        self.nc.all_engine_barrier()
        assert self.sems is not None
        popped = self.nc._tile_sem_poison_stack.pop()
        assert popped is self._sem_poison
        self.nc.clear_and_free_semaphores(list(self.sems.allocated().values()))
        self.nc.all_engine_barrier()

    tile_mod.TileContext._add_instruction = _split_add
    tile_mod.TileContext._drain_and_barrier = _drain_and_barrier
    tile_mod._wait_split_patched = True


def _build_nc():
    import concourse.bass as bass
    import concourse.tile as tile
    from concourse import mybir

    _apply_tile_patches()

    f32 = mybir.dt.float32
    bf16 = mybir.dt.bfloat16
    Alu = mybir.AluOpType
    Act = mybir.ActivationFunctionType

    nc = bass.Bass()
    from concourse.bass import _add_dep_helper

    xT = nc.dram_tensor("xT", [C, T], bf16, kind="ExternalInput")
    wnames = ["w_rf", "w_kf", "w_vf", "w_rb", "w_kb", "w_vb"]
    wdram = {
        n: nc.dram_tensor(n, [128, 4 * C], bf16, kind="ExternalInput")
        for n in wnames
    }
    wout_d = nc.dram_tensor("wout", [128, 8 * C], bf16, kind="ExternalInput")
    u_f_d = nc.dram_tensor("u_f", [C, 1], f32, kind="ExternalInput")
    u_b_d = nc.dram_tensor("u_b", [C, 1], f32, kind="ExternalInput")
    eu_f_d = nc.dram_tensor("eu_f", [C, 1], f32, kind="ExternalInput")
    eu_b_d = nc.dram_tensor("eu_b", [C, 1], f32, kind="ExternalInput")
    dec_f_d = nc.dram_tensor("dec_f", [C, 1], f32, kind="ExternalInput")
    dec_b_d = nc.dram_tensor("dec_b", [C, 1], f32, kind="ExternalInput")
    out_d = nc.dram_tensor("yT", [C, T], f32, kind="ExternalOutput")
    yst = {d: nc.dram_tensor(f"yst_{d}", [C, T], bf16) for d in ("f", "b")}

    # program-order chain for Scalar ACTs so exp/tanh batches and ln/exp
    # batches don't interleave (each interleave costs a 1.28us table load)
    act_state = {"last": None}

    def act(*args, **kwargs):
        i = nc.scalar.activation(*args, **kwargs)
        if act_state["last"] is not None:
            _add_dep_helper(i.ins, act_state["last"], False,
                            "ACT table-set program order")
        act_state["last"] = i.ins
        return i

    def act_copy(out, in_):
        i = nc.scalar.copy(out, in_)
        if act_state["last"] is not None:
            _add_dep_helper(i.ins, act_state["last"], False,
                            "ACT table-set program order")
        act_state["last"] = i.ins
        return i

    with tile.TileContext(nc) as tc:
        with (
            tc.tile_pool(name="wp", bufs=1) as wp,
            tc.tile_pool(name="cst", bufs=1) as cst,
            tc.tile_pool(name="xr", bufs=1) as xrp,
            tc.tile_pool(name="arr", bufs=2) as arrp,
            tc.tile_pool(name="chn", bufs=2) as chnp,
            tc.tile_pool(name="yc", bufs=2) as ycp,
            tc.tile_pool(name="op", bufs=2) as opp,
            tc.tile_pool(name="ps", bufs=1, space="PSUM") as psp,
        ):
            # ---- resident weights, x, constants ----
            wout = wp.tile([128, 8 * C], bf16, name="wout")
            nc.sync.dma_start(wout[:], wout_d[:])
            wt = {}
            for n in wnames:
                wt[n] = wp.tile([128, 4 * C], bf16, tag=n, name=n)
                nc.sync.dma_start(wt[n][:], wdram[n][:])
            xts = {}
            for kb in range(4):
                xts[kb] = xrp.tile([128, T], bf16, tag=f"x{kb}", name=f"x{kb}")
                nc.sync.dma_start(xts[kb][:], xT[kb * 128:(kb + 1) * 128, :])
            u_t, eu_t, dec_t = {}, {}, {}
            for cb in range(CB):
                sl = slice(cb * 128, (cb + 1) * 128)
                for d, ud, eud, dd in (("f", u_f_d, eu_f_d, dec_f_d),
                                       ("b", u_b_d, eu_b_d, dec_b_d)):
                    u_t[(d, cb)] = cst.tile([128, 1], f32, tag=f"u{d}{cb}",
                                            name=f"u{d}{cb}")
                    nc.sync.dma_start(u_t[(d, cb)][:], ud[sl, :])
                    eu_t[(d, cb)] = cst.tile([128, 1], f32, tag=f"e{d}{cb}",
                                             name=f"e{d}{cb}")
                    nc.sync.dma_start(eu_t[(d, cb)][:], eud[sl, :])
                    dec_t[(d, cb)] = cst.tile([128, 1], f32, tag=f"d{d}{cb}",
                                              name=f"d{d}{cb}")
                    nc.sync.dma_start(dec_t[(d, cb)][:], dd[sl, :])

            def run_stripe(d, cb):
                fwd = d == "f"
                wr, wk, wv = wt["w_r" + d], wt["w_k" + d], wt["w_v" + d]
                ek = arrp.tile([128, T], bf16, tag="ek", name="ek")
                ekb = arrp.tile([128, T], bf16, tag="ekb", name="ekb")
                th = arrp.tile([128, T], bf16, tag="th", name="th")
                ekv = arrp.tile([128, T], bf16, tag="ekv", name="ekv")
                chd = chnp.tile([128, T + 1], bf16, tag="chd", name="chd")
                chn = chnp.tile([128, T + 1], bf16, tag="chn", name="chn")
                ub = u_t[(d, cb)][:, 0:1]
                eu = eu_t[(d, cb)][:, 0:1]

                # ---- projections + activations, 2-wide time-tile groups
                # ekv = ek * v is a DVE mult straight out of psum_v.
                for ttg in range(NTT // 2):
                    for cls, w in (("k", wk), ("v", wv), ("r", wr)):
                        pss = {}
                        for h in range(2):
                            pss[h] = psp.tile([128, TT], f32, tag=f"p{cls}",
                                              bufs=2, name=f"p{cls}")
                        for kb in range(4):
                            wsl = w[:, kb * C + cb * 128:
                                    kb * C + cb * 128 + 128]
                            for h in range(2):
                                t0 = (2 * ttg + h) * TT
                                nc.tensor.matmul(
                                    pss[h][:], wsl, xts[kb][:, t0:t0 + TT],
                                    start=(kb == 0), stop=(kb == 3))
                        for h in range(2):
                            t0 = (2 * ttg + h) * TT
                            if cls == "k":
                                act(ek[:, t0:t0 + TT], pss[h][:], Act.Exp)
                                act(ekb[:, t0:t0 + TT], pss[h][:], Act.Exp,
                                    bias=ub)
                            elif cls == "v":
                                nc.vector.tensor_mul(ekv[:, t0:t0 + TT],
                                                     ek[:, t0:t0 + TT],
                                                     pss[h][:])
                            else:
                                act(th[:, t0:t0 + TT], pss[h][:], Act.Tanh,
                                    bias=0.0, scale=0.5)

                # ---- scans (2 chained 2048-wide per variable)
                decbc = dec_t[(d, cb)][:, 0:1].broadcast_to([128, HW])
                if fwd:
                    nc.vector.memset(chd[:, 0:1], 0.0)
                    nc.vector.memset(chn[:, 0:1], 0.0)
                    for loc in (0, HW):
                        nc.vector.tensor_tensor_scan(
                            chd[:, 1 + loc: 1 + loc + HW], decbc,
                            ek[:, loc: loc + HW], chd[:, loc: loc + 1],
                            Alu.mult, Alu.add)
                        nc.vector.tensor_tensor_scan(
                            chn[:, 1 + loc: 1 + loc + HW], decbc,
                            ekv[:, loc: loc + HW], chn[:, loc: loc + 1],
                            Alu.mult, Alu.add)
                    den_prev = chd[:, 0:T]
                    num_prev = chn[:, 0:T]
                else:
                    nc.vector.memset(chd[:, T:T + 1], 0.0)
                    nc.vector.memset(chn[:, T:T + 1], 0.0)
                    for loc in (HW, 0):
                        nc.vector.tensor_tensor_scan(
                            chd[:, loc: loc + HW][:, ::-1], decbc,
                            ek[:, loc: loc + HW][:, ::-1],
                            chd[:, loc + HW: loc + HW + 1],
                            Alu.mult, Alu.add)
                        nc.vector.tensor_tensor_scan(
                            chn[:, loc: loc + HW][:, ::-1], decbc,
                            ekv[:, loc: loc + HW][:, ::-1],
                            chn[:, loc + HW: loc + HW + 1],
                            Alu.mult, Alu.add)
                    den_prev = chd[:, 1:T + 1]
                    num_prev = chn[:, 1:T + 1]

                # ---- y stage per 1024 chunk
                #   dy = den_prev + e^u ek = den_prev + ekb     [gpsimd]
                #   ny = num_prev + e^u ekv = num_prev + ekbv   [gpsimd]
                #   y  = (ny / dy) * (1 + tanh(r/2))            [DVE 2x]
                # division via exp(-ln(dy)) on Scalar (ln/exp share one ACT
                # table). Dead array slices are recycled as outputs:
                # inv->ek, q->ekv, q2->ekb, y->th (each source slice is
                # fully consumed upstream; subtile WAR is tracked).
                for c in range(NCHK):
                    cs = slice(c * CHK, (c + 1) * CHK)
                    ekbv = ycp.tile([128, CHK], bf16, tag="ekbv",
                                    name="ekbv")
                    dy = ycp.tile([128, CHK], bf16, tag="dy", name="dy")
                    ny = ycp.tile([128, CHK], bf16, tag="ny", name="ny")
                    lnb = ycp.tile([128, CHK], f32, tag="lnb", name="lnb")
                    inv = ek[:, cs]
                    q = ekv[:, cs]
                    q2 = ekb[:, cs]
                    y = th[:, cs]
                    act(ekbv[:], ekv[:, cs], Act.Copy, scale=eu)
                    nc.gpsimd.tensor_add(dy[:], ekb[:, cs], den_prev[:, cs])
                    nc.vector.tensor_add(ny[:], ekbv[:], num_prev[:, cs])
                    act(lnb[:], dy[:], Act.Ln)
                    act(inv, lnb[:], Act.Exp, scale=-1.0)
                    nc.vector.tensor_mul(q, ny[:], inv)
                    nc.gpsimd.tensor_mul(q2, q, th[:, cs])
                    nc.gpsimd.tensor_add(y, q, q2)
                    nc.sync.dma_start(
                        yst[d][cb * 128:(cb + 1) * 128, cs], y)

            for cb in range(CB):
                run_stripe("f", cb)
            for cb in range(CB):
                run_stripe("b", cb)

            # ---- output projection: out^T[c_out, t] in (C, T) layout
            for tc_i in range(NTT):
                t0 = tc_i * TT
                yld = {}
                for j in range(8):
                    dd = "f" if j < 4 else "b"
                    cbj = j % 4
                    yld[j] = opp.tile([128, TT], bf16, tag=f"yl{j}",
                                      name=f"yl{j}")
                    nc.sync.dma_start(
                        yld[j][:],
                        yst[dd][cbj * 128:(cbj + 1) * 128, t0:t0 + TT])
                for cbo in range(CB):
                    pso = psp.tile([128, TT], f32, tag="po", bufs=2,
                                   name="pso")
                    for j in range(8):
                        blk = j * 4 + cbo
                        nc.tensor.matmul(
                            pso[:], wout[:, blk * 128:(blk + 1) * 128],
                            yld[j][:], start=(j == 0), stop=(j == 7))
                    osb = opp.tile([128, TT], f32, tag="osb", name="osb")
                    nc.scalar.copy(osb[:], pso[:])
                    nc.sync.dma_start(
                        out_d[cbo * 128:(cbo + 1) * 128, t0:t0 + TT], osb[:])

    return nc


def _host_prep(x, W_rkv, W_out, time_decay, time_first, time_decay_rev,
               time_first_rev):
    bf16 = ml_dtypes.bfloat16
    f32 = np.float32

    Wr = W_rkv.reshape(C, 2, 3, C)
    pieces = {
        "w_rf": Wr[:, 0, 0], "w_kf": Wr[:, 0, 1], "w_vf": Wr[:, 0, 2],
        "w_rb": Wr[:, 1, 0], "w_kb": Wr[:, 1, 1], "w_vb": Wr[:, 1, 2],
    }
    wmaps = {}
    for n, p in pieces.items():
        wmaps[n] = np.ascontiguousarray(
            p.reshape(4, 128, C).transpose(1, 0, 2).reshape(128, 4 * C)
        ).astype(bf16)

    # W_out blocks as stationary lhsT: block (j, cbo) = W_out rows
    # j*128:(j+1)*128, cols cbo*128:(cbo+1)*128, laid out at free offset
    # (j*4 + cbo)*128. 0.5 of the sigmoid is folded in.
    Wo = (0.5 * W_out).reshape(8, 128, 4, 128)          # (j, jp, cbo, cp)
    Wo = Wo.transpose(1, 0, 2, 3).reshape(128, 32 * 128)  # (jp, j*4+cbo, cp)
    wout = np.ascontiguousarray(Wo).astype(bf16)

    u_f = np.ascontiguousarray(time_first.reshape(C, 1)).astype(f32)
    u_b = np.ascontiguousarray(time_first_rev.reshape(C, 1)).astype(f32)
    eu_f = np.exp(time_first.astype(np.float64)).reshape(C, 1).astype(f32)
    eu_b = np.exp(time_first_rev.astype(np.float64)).reshape(C, 1).astype(f32)
    dec_f = np.exp(-np.exp(time_decay.astype(np.float64))).reshape(C, 1).astype(f32)
    dec_b = np.exp(-np.exp(time_decay_rev.astype(np.float64))).reshape(C, 1).astype(f32)

    shared = dict(wout=wout, u_f=u_f, u_b=u_b, eu_f=eu_f, eu_b=eu_b,
                  dec_f=dec_f, dec_b=dec_b, **wmaps)
    in_maps = []
    for b in range(B):
        m = dict(shared)
        m["xT"] = np.ascontiguousarray(x[b].T).astype(bf16)
        in_maps.append(m)
    return in_maps


def kernel(x, W_rkv, W_out, time_decay, time_first, time_decay_rev,
           time_first_rev, _trace=False):
    from concourse.bass_utils import run_bass_kernel_spmd

    x = np.asarray(x, dtype=np.float32)
    W_rkv = np.asarray(W_rkv, dtype=np.float32)
    W_out = np.asarray(W_out, dtype=np.float32)
    time_decay = np.asarray(time_decay, dtype=np.float32)
    time_first = np.asarray(time_first, dtype=np.float32)
    time_decay_rev = np.asarray(time_decay_rev, dtype=np.float32)
    time_first_rev = np.asarray(time_first_rev, dtype=np.float32)

    if "nc" not in _CACHE:
        _CACHE["nc"] = _build_nc()
    nc = _CACHE["nc"]

    in_maps = _host_prep(x, W_rkv, W_out, time_decay, time_first,
                         time_decay_rev, time_first_rev)
    res = run_bass_kernel_spmd(
        nc, in_maps, core_ids=list(range(B)), trace=_trace
    )
    _CACHE["last_result"] = res
    out = np.stack([
        np.ascontiguousarray(res.results[b]["yT"].astype(np.float32).T)
        for b in range(B)
    ])
    return out
